# revision 7
# baseline (speedup 1.0000x reference)
"""NT-Xent (SimCLR) contrastive loss on 8 Trainium2 NeuronCores.

Math: with x = row-normalized representation [8192, 256], tau = 0.5,
  sim = x @ x.T
  loss = (1/8192) * sum_i [ ln(sum_{j != i} exp(2 sim[i,j])) - 2 sim[i, pos(i)] ]
where pos(i) = (i + 4096) mod 8192.

Split of work:
  Host (O(N*D), numpy): row-normalize, cast to bf16, build per-core
  row-rolled copies, positive-pair dot products, final combine/ln/mean.
  Device (O(N^2*D)): the similarity matrix and row/column sums of exp(2 sim).

Symmetry: sim is symmetric, so only ~5/8 of it is computed. Each core
receives x rolled so its 1024 slab rows sit at rows 0..1023, and computes
sim[0:1024, 0:5120] (its slab rows against column blocks b=0..4; the
b=4 block is computed by both members of a (c, c+4) pair). Row sums of
exp(2 sim) cover column blocks 0..4; the missing blocks 5..7 are the
transposes of blocks 1..3 of three other cores, recovered from COLUMN
sums of exp over blocks 1..3 (computed on-device with ones-stationary
matmuls accumulating in PSUM across the 8 row-tiles). The host combines
row + column partials into full row sums S_i.

Device kernel (SPMD, identical program on all 8 cores):
  1. 10 xbar transpose-DMAs (DRAM bf16 -> SBUF) build xT chunks
     [128, 1024] for columns 0..5120 (slab tiles first).
  2. Per slab row-tile m (8): column chunks {1536,1536,1536,512} ->
     [128,1536] PSUM (2-buffer ping-pong, 6 banks) via bf16 matmuls
     (N=512, K=2x128); scalar Exp (scale=2) per chunk writes bf16 to
     SBUF with accum_out row partials -> rs [128, 32].
  3. Column sums: six ones-stationary matmuls per m over the exp'd
     [128,512] slices of blocks 1..3, accumulated over m in two PSUM
     banks at partitions {0,32,64}; emitted one m behind the main
     matmuls to keep the PE stream dense (HAM stays warm). DVE copies
     the two banks to SBUF at the end; DMA'd out with rs.
"""

import numpy as np
import ml_dtypes

import concourse.bacc as bacc
import concourse.tile as tile
from concourse import mybir
from concourse.bass_utils import run_bass_kernel_spmd

N2 = 8192            # total rows (2N)
D = 256              # feature dim
NCORES = 8
ROWS = N2 // NCORES  # 1024 slab rows per core
N = N2 // 2          # positive-pair offset
P = 128              # SBUF partitions
KC = D // P          # 2 contraction chunks of 128
CHUNK = 512          # xT tile width (= one matmul moving operand)
NXT = 10             # xT tiles per k-half (cols 0..5120)
COLS = NXT * CHUNK   # 5120 columns computed per slab row-tile
MT = ROWS // P       # 8 slab row-tiles
CW = [1536, 1536, 1536, 512]   # exp chunk widths per m (sum = COLS)
CSTART = [0, 1536, 3072, 4608]
NCH = len(CW)
NDVE = 2             # chunks whose row sum goes to DVE reduce (rest: accum_out)
# column-sum slices: six 512-wide slices covering rolled cols [1024, 4096)
# as (chunk index, offset within chunk); slice i covers cols 1024+512*i
RED = [(0, 1024), (1, 0), (1, 512), (1, 1024), (2, 0), (2, 512)]
NWARM = 16           # PE warm-up matmuls during the DMA fill (HAM un-throttle)

F32 = mybir.dt.float32
BF16 = mybir.dt.bfloat16
AF = mybir.ActivationFunctionType


def _build_kernel(tc: tile.TileContext, rs_out, cols_out, xk):
    nc = tc.nc
    with (
        tc.tile_pool(name="xt", bufs=1) as xtp,
        tc.tile_pool(name="acc", bufs=1) as accp,
        tc.tile_pool(name="esc", bufs=8) as escp,
        tc.tile_pool(name="ps", bufs=2, space="PSUM") as psp,
        tc.tile_pool(name="red", bufs=1, space="PSUM") as redp,
    ):
        xts = [[xtp.tile([P, CHUNK], BF16, tag=f"xt{k}_{j}", name=f"xt{k}_{j}")
                for j in range(NXT)] for k in range(KC)]
        rs = accp.tile([P, NCH * MT], F32, tag="rs", name="rs")
        ones = accp.tile([P, 1], BF16, tag="ones", name="ones")
        warm = accp.tile([P, 256], BF16, tag="warm", name="warm")
        colsb = accp.tile([P, 1024], F32, tag="colsb", name="colsb")
        red = [redp.tile([P, 512], F32, tag=f"red{t}", name=f"red{t}")
               for t in range(2)]
        nc.vector.memset(ones, 1.0)
        nc.vector.memset(warm, 0.0)

        # transposed bf16 keys for columns 0..5120; slab tiles first
        for j in range(NXT):
            for k in range(KC):
                nc.sync.dma_start(
                    out=xts[k][j],
                    in_=xk[k][j * CHUNK:(j + 1) * CHUNK, :],
                    transpose=True)

        # PE warm-up while the transpose DMAs fill SBUF: ~4us of matmul
        # activity flips the HAM clock gate to 8/8 before the real stream
        for _ in range(NWARM):
            nc.tensor.matmul(red[0][0:1, 0:256], ones, warm,
                             start=True, stop=True, skip_group_check=True)

        escs = {}  # (m, c) -> exp'd chunk tile

        def red_mms(m):
            for i, (c, off) in enumerate(RED):
                t, bp = i % 2, 32 * (i // 2)
                nc.tensor.matmul(
                    red[t][bp:bp + 1, :],
                    ones,
                    escs[(m, c)][:, off:off + 512],
                    start=(m == 0), stop=(m == MT - 1),
                    skip_group_check=True)

        for m in range(MT):
            mslab = (m // 4, (m % 4) * P)  # slab col m*128 -> (tile, offset)
            for c in range(NCH):
                w = CW[c]
                ps = psp.tile([P, 1536], F32, tag="ps", name="ps")
                for k in range(KC):  # k outer: stationary switches 2x/chunk
                    for half in range(w // 512):
                        jj = (CSTART[c] + half * 512) // CHUNK
                        nc.tensor.matmul(
                            ps[:, half * 512:(half + 1) * 512],
                            xts[k][mslab[0]][:, mslab[1]:mslab[1] + P],
                            xts[k][jj],
                            start=(k == 0), stop=(k == KC - 1))
                esc = escp.tile([P, 1536], BF16, tag="esc", name="esc")
                escs[(m, c)] = esc
                ridx = m * NCH + c
                if c < NDVE:  # row sum on DVE, off the scalar engine
                    nc.scalar.activation(esc[:, :w], ps[:, :w], AF.Exp,
                                         scale=2.0)
                    nc.vector.reduce_sum(rs[:, ridx:ridx + 1], esc[:, :w],
                                         axis=mybir.AxisListType.X)
                else:
                    nc.scalar.activation(
                        esc[:, :w], ps[:, :w], AF.Exp, scale=2.0,
                        accum_out=rs[:, ridx:ridx + 1])
            # column-sum matmuls one m behind: keeps the PE queue dense
            if m > 0:
                red_mms(m - 1)
        red_mms(MT - 1)

        for t in range(2):
            nc.vector.tensor_copy(colsb[:, t * 512:(t + 1) * 512], red[t])
        nc.sync.dma_start(out=rs_out, in_=rs)
        nc.sync.dma_start(out=cols_out, in_=colsb)


def build_nc():
    nc = bacc.Bacc("TRN2", target_bir_lowering=False, debug=False,
                   num_devices=NCORES)
    xk = [nc.dram_tensor(f"x{k}", [N2, P], BF16, kind="ExternalInput").ap()
          for k in range(KC)]
    rs_out = nc.dram_tensor("rs", [P, NCH * MT], F32,
                            kind="ExternalOutput").ap()
    cols_out = nc.dram_tensor("cols", [P, 1024], F32,
                              kind="ExternalOutput").ap()
    with tile.TileContext(nc) as tc:
        _build_kernel(tc, rs_out, cols_out, xk)
    nc.compile()
    return nc


_NC = None
LAST_RESULTS = None


def _make_in_maps(xb16: np.ndarray):
    in_maps = []
    for c in range(NCORES):
        xr = np.roll(xb16, -c * ROWS, axis=0)
        in_maps.append({f"x{k}": np.ascontiguousarray(xr[:, k * P:(k + 1) * P])
                        for k in range(KC)})
    return in_maps


def kernel(representation: np.ndarray, **run_kwargs) -> np.ndarray:
    global _NC, LAST_RESULTS
    rep = np.asarray(representation, dtype=np.float32)
    assert rep.shape == (N2, D)

    # host prep: normalize (f32, matching torch CosineSimilarity eps), bf16
    norms = np.maximum(np.sqrt((rep.astype(np.float64) ** 2).sum(axis=1)),
                       1e-8)
    xn = (rep / norms[:, None]).astype(np.float32)
    xb16 = xn.astype(ml_dtypes.bfloat16)

    if _NC is None:
        _NC = build_nc()
    res = run_bass_kernel_spmd(_NC, _make_in_maps(xb16),
                               core_ids=list(range(NCORES)), **run_kwargs)
    LAST_RESULTS = res

    # combine row partials (cols 0..5120 rolled) and column partials
    # (rolled cols 1024..4096, i.e. blocks b=1..3) into full row sums S
    S = np.zeros(N2, dtype=np.float64)
    for c, r in enumerate(res.results):
        rs = r["rs"].astype(np.float64).reshape(P, MT, NCH)  # [p, m, ch]
        own = rs.sum(axis=2).T.reshape(ROWS)                 # row m*128+p
        S[c * ROWS:(c + 1) * ROWS] += own
        cols = r["cols"].astype(np.float64)                  # [p, 1024]
        for i in range(6):
            colsum = cols[32 * (i // 2), (i % 2) * 512:(i % 2) * 512 + 512]
            g0 = (c * ROWS + 1024 + 512 * i) % N2
            S[g0:g0 + 512] += colsum

    # host tail: remove diagonal (as the device computed it, from bf16
    # inputs), add positive terms, final log/mean
    xb = xb16.astype(np.float64)
    ssb = (xb * xb).sum(axis=1)                  # device's sim[i,i]
    denom = S - np.exp(2.0 * ssb)
    xn64 = xn.astype(np.float64)
    pos = (xn64 * np.roll(xn64, -N, axis=0)).sum(axis=1)
    loss = (np.log(denom) - 2.0 * pos).mean()
    return np.asarray(np.float32(loss))


# revision 11
# speedup vs baseline: 1.2905x; 1.2905x over previous
"""NT-Xent (SimCLR) contrastive loss on 8 Trainium2 NeuronCores.

Math: with x = row-normalized representation [8192, 256], tau = 0.5,
  sim = x @ x.T
  loss = (1/8192) * sum_i [ ln(sum_{j != i} exp(2 sim[i,j])) - 2 sim[i, pos(i)] ]
where pos(i) = (i + 4096) mod 8192.

Split of work:
  Host (O(N*D), numpy): row-normalize, cast to bf16, build per-core
  row-rolled copies, positive-pair dot products, final combine/ln/mean.
  Device (O(N^2*D)): the similarity matrix and row/column sums of exp(2 sim).

Symmetry: sim is symmetric, so only ~5/8 of it is computed. Each core
receives x rolled so its 1024 slab rows sit at rows 0..1023, and computes
sim[0:1024, 0:5120] (its slab rows against column blocks b=0..4; the
b=4 block is computed by both members of a (c, c+4) pair). Row sums of
exp(2 sim) cover column blocks 0..4; the missing blocks 5..7 are the
transposes of blocks 1..3 of three other cores, recovered from COLUMN
sums of exp over blocks 1..3 (computed on-device with ones-stationary
matmuls accumulating in PSUM across the 8 row-tiles). The host combines
row + column partials into full row sums S_i.

Device kernel (SPMD, identical program on all 8 cores):
  1. 10 xbar transpose-DMAs (DRAM bf16 -> SBUF) build xT chunks
     [128, 1024] for columns 0..5120 (slab tiles first).
  2. Per slab row-tile m (8): column chunks {1536,1536,1536,512} ->
     [128,1536] PSUM (2-buffer ping-pong, 6 banks) via bf16 matmuls
     (N=512, K=2x128); scalar Exp (scale=2) per chunk writes bf16 to
     SBUF with accum_out row partials -> rs [128, 32].
  3. Column sums: six ones-stationary matmuls per m over the exp'd
     [128,512] slices of blocks 1..3, accumulated over m in two PSUM
     banks at partitions {0,32,64}; emitted one m behind the main
     matmuls to keep the PE stream dense (HAM stays warm). DVE copies
     the two banks to SBUF at the end; DMA'd out with rs.
"""

import numpy as np
import ml_dtypes

import concourse.bacc as bacc
import concourse.tile as tile
from concourse import mybir
from concourse.bass_utils import run_bass_kernel_spmd

N2 = 8192            # total rows (2N)
D = 256              # feature dim
NCORES = 8
ROWS = N2 // NCORES  # 1024 slab rows per core
N = N2 // 2          # positive-pair offset
P = 128              # SBUF partitions
KC = D // P          # 2 contraction chunks of 128
MT = ROWS // P       # 8 slab row-tiles
CW = [1536, 1536, 1536, 512]   # chunk widths (xT tiles align with chunks)
CSTART = [0, 1536, 3072, 4608]
COLS = sum(CW)       # 5120 columns computed per slab row-tile
NCH = len(CW)
NDVE = 2             # chunks whose row sum goes to DVE reduce (rest: accum_out)
# column-sum slices: six 512-wide slices covering rolled cols [1024, 4096)
# as (chunk index, offset within chunk); slice i covers cols 1024+512*i
RED = [(0, 1024), (1, 0), (1, 512), (1, 1024), (2, 0), (2, 512)]

F32 = mybir.dt.float32
BF16 = mybir.dt.bfloat16
AF = mybir.ActivationFunctionType


def _build_kernel(tc: tile.TileContext, rs_out, cols_out, xk):
    nc = tc.nc
    with (
        tc.tile_pool(name="xt", bufs=1) as xtp,
        tc.tile_pool(name="acc", bufs=1) as accp,
        tc.tile_pool(name="esc01", bufs=MT * NDVE) as escp01,
        tc.tile_pool(name="esc2", bufs=MT) as escp2,
        tc.tile_pool(name="esc3", bufs=2) as escp3,
        tc.tile_pool(name="ps", bufs=2, space="PSUM") as psp,
        tc.tile_pool(name="red", bufs=1, space="PSUM") as redp,
    ):
        xts = [[xtp.tile([P, CW[c]], BF16, tag=f"xt{k}_{c}", name=f"xt{k}_{c}")
                for c in range(NCH)] for k in range(KC)]
        rs = accp.tile([P, NCH * MT], F32, tag="rs", name="rs")
        ones = accp.tile([P, 1], BF16, tag="ones", name="ones")
        dume = accp.tile([P, 1], BF16, tag="dume", name="dume")
        colsb = accp.tile([P, 1024], F32, tag="colsb", name="colsb")
        red = [redp.tile([P, 512], F32, tag=f"red{t}", name=f"red{t}")
               for t in range(2)]
        nc.vector.memset(ones, 1.0)
        # pull the exp ACT_TABLE_LOAD into the DMA fill window
        nc.scalar.activation(dume, ones, AF.Exp, scale=1.0)

        # transposed bf16 keys, one xT tile per column chunk; chunk 0 first
        for c in range(NCH):
            for k in range(KC):
                nc.sync.dma_start(
                    out=xts[k][c],
                    in_=xk[k][CSTART[c]:CSTART[c] + CW[c], :],
                    transpose=True)

        escs = {}  # (m, c) -> exp'd chunk tile

        # chunk-outer sweeps: the whole first sweep (48 matmuls) depends only
        # on xT tile 0, giving a dense PE stream that warms HAM by itself
        for c in range(NCH):
            w = CW[c]
            for m in range(MT):
                ps = psp.tile([P, 1536], F32, tag="ps", name="ps")
                for k in range(KC):  # k outer: stationary switches 2x/chunk
                    for half in range(w // 512):
                        nc.tensor.matmul(
                            ps[:, half * 512:(half + 1) * 512],
                            xts[k][0][:, m * P:(m + 1) * P],
                            xts[k][c][:, half * 512:(half + 1) * 512],
                            start=(k == 0), stop=(k == KC - 1))
                pool = (escp01 if c < NDVE else
                        escp2 if c == 2 else escp3)
                esc = pool.tile([P, 1536], BF16, tag="esc", name="esc")
                escs[(m, c)] = esc
                ridx = m * NCH + c
                if c < NDVE:  # row sum on DVE, off the scalar engine
                    nc.scalar.activation(esc[:, :w], ps[:, :w], AF.Exp,
                                         scale=2.0)
                    nc.vector.reduce_sum(rs[:, ridx:ridx + 1], esc[:, :w],
                                         axis=mybir.AxisListType.X)
                else:
                    nc.scalar.activation(
                        esc[:, :w], ps[:, :w], AF.Exp, scale=2.0,
                        accum_out=rs[:, ridx:ridx + 1])
                # column-sum matmuls ride along with the last (cheap) sweep
                if c == NCH - 1:
                    for i, (rc, off) in enumerate(RED):
                        t, bp = i % 2, 32 * (i // 2)
                        nc.tensor.matmul(
                            red[t][bp:bp + 1, :],
                            ones,
                            escs[(m, rc)][:, off:off + 512],
                            start=(m == 0), stop=(m == MT - 1),
                            skip_group_check=True)

        for t in range(2):
            nc.vector.tensor_copy(colsb[:, t * 512:(t + 1) * 512], red[t])
        nc.sync.dma_start(out=rs_out, in_=rs)
        nc.sync.dma_start(out=cols_out, in_=colsb)


def build_nc():
    nc = bacc.Bacc("TRN2", target_bir_lowering=False, debug=False,
                   num_devices=NCORES)
    xk = [nc.dram_tensor(f"x{k}", [N2, P], BF16, kind="ExternalInput").ap()
          for k in range(KC)]
    rs_out = nc.dram_tensor("rs", [P, NCH * MT], F32,
                            kind="ExternalOutput").ap()
    cols_out = nc.dram_tensor("cols", [P, 1024], F32,
                              kind="ExternalOutput").ap()
    with tile.TileContext(nc) as tc:
        _build_kernel(tc, rs_out, cols_out, xk)
    nc.compile()
    return nc


_NC = None
LAST_RESULTS = None


def _make_in_maps(xb16: np.ndarray):
    in_maps = []
    for c in range(NCORES):
        xr = np.roll(xb16, -c * ROWS, axis=0)
        in_maps.append({f"x{k}": np.ascontiguousarray(xr[:, k * P:(k + 1) * P])
                        for k in range(KC)})
    return in_maps


def kernel(representation: np.ndarray, **run_kwargs) -> np.ndarray:
    global _NC, LAST_RESULTS
    rep = np.asarray(representation, dtype=np.float32)
    assert rep.shape == (N2, D)

    # host prep: normalize (f32, matching torch CosineSimilarity eps), bf16
    norms = np.maximum(np.sqrt((rep.astype(np.float64) ** 2).sum(axis=1)),
                       1e-8)
    xn = (rep / norms[:, None]).astype(np.float32)
    xb16 = xn.astype(ml_dtypes.bfloat16)

    if _NC is None:
        _NC = build_nc()
    res = run_bass_kernel_spmd(_NC, _make_in_maps(xb16),
                               core_ids=list(range(NCORES)), **run_kwargs)
    LAST_RESULTS = res

    # combine row partials (cols 0..5120 rolled) and column partials
    # (rolled cols 1024..4096, i.e. blocks b=1..3) into full row sums S
    S = np.zeros(N2, dtype=np.float64)
    for c, r in enumerate(res.results):
        rs = r["rs"].astype(np.float64).reshape(P, MT, NCH)  # [p, m, ch]
        own = rs.sum(axis=2).T.reshape(ROWS)                 # row m*128+p
        S[c * ROWS:(c + 1) * ROWS] += own
        cols = r["cols"].astype(np.float64)                  # [p, 1024]
        for i in range(6):
            colsum = cols[32 * (i // 2), (i % 2) * 512:(i % 2) * 512 + 512]
            g0 = (c * ROWS + 1024 + 512 * i) % N2
            S[g0:g0 + 512] += colsum

    # host tail: remove diagonal (as the device computed it, from bf16
    # inputs), add positive terms, final log/mean
    xb = xb16.astype(np.float64)
    ssb = (xb * xb).sum(axis=1)                  # device's sim[i,i]
    denom = S - np.exp(2.0 * ssb)
    xn64 = xn.astype(np.float64)
    pos = (xn64 * np.roll(xn64, -N, axis=0)).sum(axis=1)
    loss = (np.log(denom) - 2.0 * pos).mean()
    return np.asarray(np.float32(loss))
